# revision 13
# baseline (speedup 1.0000x reference)
"""VQ codebook soft-assignment (Student-t, alpha=1) for Trainium2.

q[b,k] = w / sum_k w,  w = 1 / (1 + ||x_b - c_k||^2)
       = 1 / (s_b + t_k - 2 x.c),  s_b = 1 + ||x_b||^2, t_k = ||c_k||^2

Data-parallel over 8 NeuronCores: x sharded along batch; the replicated
centroid matrix is host-prepped into the layout the PE array wants
(-2*c^T as bf16 d-major chunks, plus a bias operand carrying [ones; ||c||^2]).

Per-core device work (B_CORE=2048, K=2048, D=512):
  - load x f32; ACT Square+accum -> ||x_b||^2; DVE cast x->bf16;
    DMA-xbar transpose to d-major layout (a tiny SP "observer" copy first
    keeps each XPOSE instruction at <=1 sync wait - ISA struct limit)
  - s_b row via PE transpose + SBUF flatten DMA into a zero-padded fp32
    bias operand; rank-2 fp32 bias matmul accumulates s_b + t_k into PSUM
    on top of the 16 bf16 dot matmuls per b-tile -> PSUM = denom
  - custom DVE op RECIP_QUAD_ACC: q_u = 1/denom via exponent-flip seed +
    quadratic minimax poly (max rel err ~5e-5), accum_out = rowsum
  - DVE reciprocal of rowsum [128,1]; ACT Copy with per-partition scale
  - DMA out f32
"""

import numpy as np

B, D, K = 16384, 512, 2048
N_CORES = 8
B_CORE = B // N_CORES  # 2048
P = 128
NB = B_CORE // P       # 16 b-tiles per core
ND = D // P            # 4 d-chunks
KS = 512               # k-slice width (one PSUM bank of f32)
NK = K // KS           # 4 k-slices

# Quadratic minimax seed for 1/x via t = x * bitcast(~bits(x)) in [-4.5, -4]:
# 1/t ~ A0 + A1*t + A2*t^2  (max rel err ~5.1e-5 over the interval)
A0 = -0.70710608
A1 = -0.16652187
A2 = -0.01306054

_OP_NAME = "RECIP_QUAD_ACC_ANT"


def _register_recip_op():
    """Register the fused reciprocal+rowsum custom DVE op (idempotent)."""
    from operator import add

    import concourse.dve_ops as dve_ops
    from concourse.dve_spec import (
        AluOp,
        Bin,
        C0,
        C1,
        C2,
        Spec,
        Src0,
        Zero,
        _has_src1,
        lower,
    )
    from concourse.dve_uop import DveOpSpec

    for op in dve_ops.OPS:
        if op.name == _OP_NAME:
            return op

    _n = Bin(AluOp.BITWISE_NOT, Src0, Src0)
    _t = Src0 * _n
    body = ((_t * C2 + C1) * _t + C0) * _n

    def _ref(in0, in1, c0, c1, c2):
        x = np.ascontiguousarray(in0, dtype=np.float32)
        n = (~x.view(np.int32)).view(np.float32)
        t = x * n
        y = (((t * c2 + c1) * t + c0) * n).astype(np.float32)
        return y, y.reshape(y.shape[0], -1).sum(axis=-1, keepdims=True)

    spec = Spec(body=body, accum=add, accum_init=Zero, reference=_ref)
    opcode = dve_ops._CUSTOM_DVE_ROW_BASE + len(dve_ops.OPS)
    assert opcode < 0x20
    shas = {}
    for ver in ("v3", "v4"):
        s = DveOpSpec(
            name=_OP_NAME,
            opcode=opcode,
            uops=lower(spec, ver=ver),
            rd1_en=_has_src1(spec),
        )
        shas[ver] = s.sha(ver)
    op = dve_ops.DveOp(_OP_NAME, spec, subdim=False, uops_sha=shas)
    dve_ops.OPS.append(op)
    dve_ops._SUB_OPCODE_FOR_NAME[_OP_NAME] = opcode
    dve_ops.CUSTOM_DVE_SPECS[_OP_NAME] = spec
    return op


def prep_centroid_inputs(centroids: np.ndarray):
    """Host-side weight prep for the replicated centroid matrix.

    Returns
      ct:      [ND, P, K] bf16  chunks of (-2 c)^T (d-major)
      bias_mv: [P, K] f32       row0 = ones, row1 = ||c_k||^2, rest zero
    """
    import ml_dtypes

    c = np.ascontiguousarray(centroids, dtype=np.float32)
    cn2 = (-2.0 * c).astype(ml_dtypes.bfloat16)  # [K, D]
    ct = np.ascontiguousarray(cn2.T.reshape(ND, P, K))
    bias_mv = np.zeros((P, K), dtype=np.float32)
    bias_mv[0, :] = 1.0
    bias_mv[1, :] = (c.astype(np.float64) ** 2).sum(axis=1).astype(np.float32)
    return ct, bias_mv


def emit_kernel(ctx, tc, q_d, x_d, ct_d, bmv_d):
    """Emit the per-core kernel body into TileContext tc.

    q_d: [B_CORE, K] f32 out; x_d: [B_CORE, D] f32;
    ct_d: [ND, P, K] bf16; bmv_d: [P, K] f32.
    """
    import concourse.mybir as mybir
    from concourse.bass import ts
    from concourse.masks import make_identity

    nc = tc.nc
    f32 = mybir.dt.float32
    bf16 = mybir.dt.bfloat16
    AF = mybir.ActivationFunctionType
    recip_op = _register_recip_op()

    KH = 2 * KS  # 1024: half-tile of k (2 PSUM banks)

    const = ctx.enter_context(tc.tile_pool(name="const", bufs=1))
    ld = ctx.enter_context(tc.tile_pool(name="ld", bufs=16))
    sq = ctx.enter_context(tc.tile_pool(name="sq", bufs=2))
    bfp = ctx.enter_context(tc.tile_pool(name="bfp", bufs=3))
    psum = ctx.enter_context(tc.tile_pool(name="psum", bufs=2, space="PSUM"))
    tpp = ctx.enter_context(tc.tile_pool(name="tpp", bufs=2, space="PSUM"))
    spp = ctx.enter_context(tc.tile_pool(name="spp", bufs=2, space="PSUM"))
    qu_p = ctx.enter_context(tc.tile_pool(name="qu", bufs=3))
    qo_p = ctx.enter_context(tc.tile_pool(name="qo", bufs=3))
    sm = ctx.enter_context(tc.tile_pool(name="sm", bufs=8))

    xT = const.tile([P, ND, B_CORE], bf16)      # x^T, d-major
    cT = const.tile([P, ND, K], bf16)           # (-2 c)^T, d-major
    bias_mv = const.tile([P, K], f32)           # [1; t_k; 0...]
    x2c = const.tile([P, NB], f32)              # ||x_b||^2, column layout
    bias_st = const.tile([P, B_CORE], f32)      # [s_b; 1; 0...]
    ident_f = const.tile([P, P], f32)
    ident_b = const.tile([P, P], bf16)

    make_identity(nc, ident_f[:])
    make_identity(nc, ident_b[:])
    nc.vector.memset(bias_st[:], 0.0)
    nc.vector.memset(bias_st[0:2, :], 1.0)

    # centroid operands arrive pre-packed
    for dc in range(ND):
        nc.sync.dma_start(cT[:, dc, :], ct_d[dc])
    nc.sync.dma_start(bias_mv[:], bmv_d[:])

    # ---- main loop over b-tiles ----
    # Per-tile prologue (load/cast/row-norm/PE-transpose) is interleaved with
    # the matmul groups so the PE never sits behind a global barrier. All
    # DMAs carry at most one semaphore wait (the DMA ISA struct's limit):
    # loads wait only on the DVE cast (slot WAR), stores only on ACT scale.
    for j in range(NB):
        # load + cast + row-norm (Square reads the bf16 so x2 rounding is
        # consistent with the matmul operand)
        xt = ld.tile([P, D], f32, tag="ld")
        nc.sync.dma_start(xt[:], x_d[ts(j, P), :])
        xb = bfp.tile([P, D], bf16, tag="bfp")
        nc.vector.tensor_copy(xb[:], xt[:])
        st = sq.tile([P, D], f32, tag="sq")
        nc.scalar.activation(st[:], xb[:], AF.Square, accum_out=x2c[:, j : j + 1])

        # s_b row: PE-transpose the [128,1] column into a [1,128] row and
        # fold the +1 into the ACT copyback bias
        ps = spp.tile([1, P], f32, tag="sp")
        nc.tensor.transpose(ps[:, :], x2c[:, j : j + 1], ident_f[:])
        nc.scalar.activation(bias_st[0:1, ts(j, P)], ps[:, :], AF.Copy, bias=1.0)

        # x^T via PE transposes (alternate DVE/ACT copybacks for balance)
        for dc in range(ND):
            tp = tpp.tile([P, P], bf16, tag="tp")
            nc.tensor.transpose(tp[:, :], xb[:, ts(dc, P)], ident_b[:])
            if dc % 2 == 0:
                nc.vector.tensor_copy(xT[:, dc, ts(j, P)], tp[:, :])
            else:
                nc.scalar.copy(xT[:, dc, ts(j, P)], tp[:, :])

        qu = qu_p.tile([P, K], f32, tag="qu")
        rs01 = []
        for h in range(2):
            pt = psum.tile([P, KH], f32, tag="pt")
            for ks2 in range(2):
                ks = 2 * h + ks2
                for dc in range(ND):
                    nc.tensor.matmul(
                        pt[:, ts(ks2, KS)],
                        xT[:, dc, ts(j, P)],
                        cT[:, dc, ts(ks, KS)],
                        start=(dc == 0),
                        stop=False,
                    )
                # rank-2 fp32 bias matmul (zero-padded to K=128): += s_b + t_k
                nc.tensor.matmul(
                    pt[:, ts(ks2, KS)],
                    bias_st[:, ts(j, P)],
                    bias_mv[:, ts(ks, KS)],
                    start=False,
                    stop=True,
                )
            rs = sm.tile([P, 1], f32, tag=f"rs{h}")
            nc.vector._custom_dve(
                recip_op,
                out=qu[:, ts(h, KH)],
                in0=pt[:],
                s0=A0,
                s1=A1,
                imm2=A2,
                accum_out=rs[:],
            )
            rs01.append(rs)
        rst = sm.tile([P, 1], f32, tag="rst")
        nc.vector.tensor_tensor(
            rst[:], rs01[0][:], rs01[1][:], mybir.AluOpType.add
        )
        rr = sm.tile([P, 1], f32, tag="rr")
        nc.vector.reciprocal(rr[:], rst[:])
        qo = qo_p.tile([P, K], f32, tag="qo")
        nc.scalar.activation(qo[:], qu[:], AF.Copy, bias=0.0, scale=rr[:])
        nc.sync.dma_start(q_d[ts(j, P), :], qo[:])


def build_bass(repeat: int = 1):
    """Build the single-core Bass module (same NEFF runs SPMD on all cores).

    repeat > 1 wraps the body in a device-side For loop (identical I/O,
    repeat x the work) -- used only for execution-time measurement.
    """
    from contextlib import ExitStack

    import concourse.mybir as mybir
    import concourse.tile as tile
    from concourse import bacc

    f32 = mybir.dt.float32
    bf16 = mybir.dt.bfloat16
    nc = bacc.Bacc("TRN2", target_bir_lowering=False, debug=False)
    x_d = nc.dram_tensor("x", (B_CORE, D), f32, kind="ExternalInput").ap()
    ct_d = nc.dram_tensor("ct", (ND, P, K), bf16, kind="ExternalInput").ap()
    bmv_d = nc.dram_tensor("bias_mv", (P, K), f32, kind="ExternalInput").ap()
    q_d = nc.dram_tensor("q", (B_CORE, K), f32, kind="ExternalOutput").ap()
    with tile.TileContext(nc) as tc:
        with ExitStack() as ctx:
            if repeat == 1:
                emit_kernel(ctx, tc, q_d, x_d, ct_d, bmv_d)
            else:
                with tc.For_i(0, repeat, 1):
                    emit_kernel(ctx, tc, q_d, x_d, ct_d, bmv_d)
    nc.compile()
    return nc


_BUILT = None


def _get_built():
    global _BUILT
    if _BUILT is None:
        _BUILT = build_bass()
    return _BUILT


def make_in_maps(x: np.ndarray, centroids: np.ndarray):
    x = np.ascontiguousarray(x, dtype=np.float32)
    ct, bias_mv = prep_centroid_inputs(centroids)
    return [
        {
            "x": np.ascontiguousarray(x[i * B_CORE : (i + 1) * B_CORE]),
            "ct": ct,
            "bias_mv": bias_mv,
        }
        for i in range(N_CORES)
    ]


def kernel(x: np.ndarray, centroids: np.ndarray) -> np.ndarray:
    import concourse.bass_utils as bass_utils

    assert x.shape == (B, D) and centroids.shape == (K, D)
    nc = _get_built()
    in_maps = make_in_maps(x, centroids)
    res = bass_utils.run_bass_kernel_spmd(nc, in_maps, core_ids=list(range(N_CORES)))
    return np.concatenate([r["q"] for r in res.results], axis=0)


if __name__ == "__main__":
    import reference

    inputs = reference.setup_inputs()
    expected = np.asarray(reference.reference(**inputs))
    actual = kernel(**{k: np.asarray(v) for k, v in inputs.items()})
    err = np.abs(actual - expected).max() / np.abs(expected).max()
    rel = np.linalg.norm(actual - expected) / np.linalg.norm(expected)
    print(f"max-abs-rel: {err:.3e}  fro-rel: {rel:.3e}")
